# revision 53
# baseline (speedup 1.0000x reference)
"""Trainium2 Bass kernel for nn_DecoderBlock_Mamba (AxialDW conv + 1x1 conv +
BN + ReLU + LN + Mamba selective scan + residual).

Sharding: 8 cores = (batch b in 0..3) x (state-half sigma in {0,1}).
Each core runs the full per-image pipeline for its batch element but only 8 of
the 16 SSM states; partial y is AllReduce'd within core pairs, post-stack is
computed redundantly on both cores of a pair.

v2 design notes (vs baseline):
- front conv reads a single zero-padded image (66x66) with shifted APs; no
  host-side shifted-image copies (1 small DMA instead of 5).
- causal depthwise conv1d folded into in_proj: 4 tap matmuls on PE with
  host-precomposed (w_k[d] * ip_w[d,c] * ln_g[c]) weights; the xm tensor is
  never materialized and the DVE chain is gone.
- dt path: dt_proj @ x_proj_dt precomposed on host -> one matmul + exp + ln.
- all activations stay in the natural_log_exp table set except one Silu
  phase (ZS and XC silus adjacent) -> 2 table loads instead of 20.
- B_j broadcasts via DMA (partition-stride-0 source) into bf16 SBUF so the
  dbx multiply runs at the DVE 2x bf16 rate; C_j broadcasts are a tunable
  mix of DMA route and PE-matmul route.
- scans split between DVE and Pool (gpsimd) engines; y accumulated per
  L-half into parked PSUM via identity matmuls.

Self-contained: hardcodes all shapes; no sibling imports.
"""
import numpy as np

C = 64
DI = 128
DS = 16
DR = 4
B = 4
H = 64
W = 64
L = H * W
NS = 8            # states per core
NCORES = 8
CH = 512
NCH = L // CH
LH = L // 2       # half-L for scan passes
EPS = 1e-5
WP = 66           # padded image row width
XPL = WP * WP     # 4356

# ---- tunables: engine routing ----
# walrus legality: scans + PSUM reads are DVE-only; Pool does SBUF TT/ts only.
# route per state for the B-side dbx mult and C-side y mult:
#   'dve': DVE stt directly from broadcast PSUM f32 (1.04/col)
#   'ap':  ACT copies bcast PSUM -> SBUF bf16, Pool TT bf16 (0.83 + 1.98)
#   'ad':  ACT copies bcast PSUM -> SBUF bf16, DVE TT bf16 (0.83 + 0.52)
B_ROUTE = ['dve','ad','dve','ad','dve','ad','dve','ad','dve','ad','dve','ad','dve','ad','dve','ad']
C_ROUTE = ['ad','dve','ad','ap','ad','dve','ad','ap','ad','ap','ad','dve','ad','ap','ad','dve']

_cached = {}


def _build_program(sim=False, phases=99):
    import concourse.bass as bass
    import concourse.bacc as bacc
    import concourse.mybir as mybir
    import concourse.tile as tile

    dt = mybir.dt
    f32 = dt.float32
    bf16 = dt.bfloat16
    Act = mybir.ActivationFunctionType
    Alu = mybir.AluOpType
    Axis = mybir.AxisListType

    # Patch the activation-table list seen by the act-load placement pass:
    # drop exp/ln from every set except natural_log_exp_and_others so the
    # greedy chooser keeps Exp+Ln in ONE table (indices/names unchanged, so
    # walrus still loads the right physical tables).
    import concourse.hw_specs as _hws
    _orig_gat = _hws.get_activation_tables.__wrapped__ if hasattr(
        _hws.get_activation_tables, "__wrapped__") else None
    if not getattr(bacc, "_act_tables_patched", False):
        _inner = bacc.get_activation_tables

        def _patched(arch):
            t = dict(_inner(arch))
            exp = mybir.ActivationFunctionType.Exp
            ln = mybir.ActivationFunctionType.Ln
            out = {}
            for name, funcs in t.items():
                if name != "natural_log_exp_and_others":
                    funcs = funcs - {exp, ln}
                out[name] = funcs
            return out

        bacc.get_activation_tables = _patched
        bacc._act_tables_patched = True

    nc = bacc.Bacc(None, target_bir_lowering=False)

    def din(name, shape, dtype=f32):
        return nc.dram_tensor(name, shape, dtype, kind="ExternalInput")

    xpad_d = din("xpad", [C, XPL], bf16)
    cf32_d = din("cf32", [128, 16])
    cbf_d = din("cbf", [128, 3344], bf16)
    out_d = nc.dram_tensor("out_f", [C, L], f32, kind="ExternalOutput")

    groups = [[0, 1], [2, 3], [4, 5], [6, 7]]

    with tile.TileContext(nc) as tc:
        with (
            tc.tile_pool(name="dram", bufs=1, space="DRAM") as dpool,
            tc.tile_pool(name="const", bufs=1) as cpool,
            tc.tile_pool(name="big", bufs=1) as bpool,
            tc.tile_pool(name="sm", bufs=3) as spool,
            tc.tile_pool(name="da", bufs=3) as dapool,
            tc.tile_pool(name="dbx", bufs=3) as dbxpool,
            tc.tile_pool(name="hp", bufs=1) as hpool,
            tc.tile_pool(name="tp", bufs=3) as tmppool,
            tc.tile_pool(name="psA", bufs=3, space="PSUM") as psA,
            tc.tile_pool(name="psZ", bufs=1, space="PSUM") as psZ,
        ):
            # ---- constants ----
            cf = cpool.tile([128, 16], f32)
            cb = cpool.tile([128, 3344], bf16)
            nc.sync.dma_start(cf[:], cf32_d[:])
            nc.sync.dma_start(cb[:], cbf_d[:])
            bn_b = cf[0:C, 0:1]
            ip_bz = cf[:, 1:2]
            cd_be = cf[:, 2:3]
            dt_b = cf[:, 3:4]
            a_sc = cf[:, 4:12]
            Dp = cf[:, 12:13]
            ident = cb[:, 0:128]
            taps5 = cb[0:C, 128:448]           # 5 x [64, 64]
            ipz = cb[0:C, 448:576]             # [64, 128]
            c1d = cb[0:C, 576:1088]            # 4 x [64, 128]
            Mdt = cb[:, 1088:1216]             # [128, 128]
            bcT = cb[:, 1216:1232]             # [128, 16]
            opT = cb[:, 1232:1296]             # [128, 64]
            brepT = cb[:, 1296:2320]           # 8 x [128, 128] replicated B rows
            crepT = cb[:, 2320:3344]           # 8 x [128, 128] replicated C rows

            # warm ACT's wait slots on const DMAs
            warm = cpool.tile([128, 1], f32, tag="warm")
            nc.scalar.activation(warm[:], cf[:, 0:1], Act.Copy)
            warm2 = cpool.tile([128, 1], bf16, tag="warm2")
            nc.scalar.activation(warm2[:], cb[:, 0:1], Act.Copy)

            ONES = cpool.tile([128, 512], bf16, tag="ones")
            nc.gpsimd.memset(ONES[:], 1.0)

            # ---- persistent activations ----
            XP = bpool.tile([C, XPL], bf16, name="XP")
            nc.sync.dma_start(XP[:, 0:XPL // 2], xpad_d[:, 0:XPL // 2])
            nc.sync.dma_start(XP[:, XPL // 2:], xpad_d[:, XPL // 2:])
            XP3 = XP[:].rearrange("p (r w) -> p r w", w=WP)

            SEQ = bpool.tile([C, L], bf16)
            HN = bpool.tile([C, 4 + L], bf16, name="HN")
            nc.vector.tensor_scalar_mul(HN[:, 0:4], cf[0:C, 0:4], 0.0)
            ZS = bpool.tile([DI, L], bf16)
            XC = bpool.tile([DI, L], bf16)
            DT = bpool.tile([DI, L], bf16)
            U = bpool.tile([DI, L], bf16)
            CAR = bpool.tile([128, NS], f32, name="CAR")
            YSUM = bpool.tile([DI, L], bf16, name="YSUM")

            # ---- front conv: 5 taps from padded image + BN + ReLU ----
            # XP3 index: row r_pad = abs_row + 1, col w_pad = w + 1
            offs = [(1, 1), (0, 1), (2, 1), (1, 0), (1, 2)]  # ctr, up, dn, lf, rt
            for ci in range(NCH):
                pc = psA.tile([C, CH], f32, tag="mm")
                for k, (ro, co) in enumerate(offs):
                    rhs = XP3[:, ro + 8 * ci: ro + 8 * ci + 8, co:co + 64]
                    nc.tensor.matmul(pc[:], taps5[:, k * 64:(k + 1) * 64], rhs,
                                     start=(k == 0), stop=(k == 4))
                nc.scalar.activation(SEQ[:, ci * CH:(ci + 1) * CH], pc[:],
                                     Act.Relu, bias=bn_b)

            # ---- LayerNorm over channels (rstd via ln+exp, stay in nle set) ----
            HN0 = bpool.tile([128, L // 2], bf16, name="HN0", tag="HN0")
            HNT = bpool.tile([128, L // 2], bf16, name="HNT", tag="HNT")
            VARS = spool.tile([128, 32], f32, tag="VARS")
            LNV = spool.tile([128, 32], f32, tag="LNV")
            RSTD = spool.tile([128, 32], f32, tag="RSTD")
            NG = L // 512
            with tc.high_priority():
              for g in range(NG):
                tps4 = psA.tile([128, 4, C], bf16, tag="mm")
                for k in range(4):
                    blk = g * 4 + k
                    nc.tensor.transpose(tps4[:, k, :],
                                        SEQ[:, blk * 128:(blk + 1) * 128],
                                        ident[0:C, 0:C])
                mu4 = spool.tile([128, 4], f32, tag="mu4")
                nc.vector.tensor_reduce(mu4[:], tps4[:], Axis.X, Alu.add)
                mun4 = spool.tile([128, 4], f32, tag="mun4")
                nc.vector.tensor_scalar_mul(mun4[:], mu4[:], 1.0 / C)
                h04 = HN0[:, g * 256:(g + 1) * 256].rearrange(
                    "p (b c) -> p b c", b=4)
                nc.vector.tensor_tensor(h04, tps4[:],
                                        mun4[:].to_broadcast((128, 4, C)),
                                        op=Alu.subtract)
                sq4 = spool.tile([128, 4, C], bf16, tag="sq4")
                nc.gpsimd.tensor_mul(sq4[:], h04, h04)
                ssq4 = spool.tile([128, 4], f32, tag="ssq4")
                nc.vector.tensor_reduce(ssq4[:], sq4[:], Axis.X, Alu.add)
                nc.vector.tensor_scalar(VARS[:, g * 4:(g + 1) * 4], ssq4[:],
                                        1.0 / C, EPS,
                                        op0=Alu.mult, op1=Alu.add)
                nc.scalar.activation(LNV[:, g * 4:(g + 1) * 4],
                                     VARS[:, g * 4:(g + 1) * 4], Act.Ln)
                nc.scalar.activation(RSTD[:, g * 4:(g + 1) * 4],
                                     LNV[:, g * 4:(g + 1) * 4],
                                     Act.Exp, scale=-0.5)
            with tc.high_priority():
             for g in range(NG):
                hnT4 = HNT[:, g * 256:(g + 1) * 256].rearrange(
                    "p (b c) -> p b c", b=4)
                nc.gpsimd.tensor_tensor(
                    hnT4, HN0[:, g * 256:(g + 1) * 256].rearrange(
                        "p (b c) -> p b c", b=4),
                    RSTD[:, g * 4:(g + 1) * 4].to_broadcast((128, 4, C)),
                    op=Alu.mult)
                tb4 = psA.tile([C, 4, 128], bf16, tag="mm")
                for k in range(4):
                    blk = g * 4 + k
                    nc.tensor.transpose(tb4[:, k, :],
                                        HNT[:, blk * C:(blk + 1) * C],
                                        ident)
                nc.scalar.copy(HN[:, 4 + g * CH: 4 + (g + 1) * CH],
                               tb4[:].rearrange("p a b -> p (a b)"))

            # ---- in_proj z + conv1d taps (PE) + one Silu phase ----
            # group order ends at g1 so dt (which starts at g1) waits for the
            # last silu (no table ping-pong) while DT half-0 finishes first
            NG2 = L // 1024
            for g in ([0, 1, 2, 3] if phases >= 1 else []):
                zp = psA.tile([DI, 1024], f32, tag="mm")
                cp = psA.tile([DI, 1024], f32, tag="mm")
                for q in range(2):
                    sl = slice(g * 1024 + q * CH, g * 1024 + (q + 1) * CH)
                    nc.tensor.matmul(zp[:, q * CH:(q + 1) * CH], ipz,
                                     HN[:, 4 + g * 1024 + q * CH:
                                        4 + g * 1024 + (q + 1) * CH],
                                     start=True, stop=True)
                    for k in range(4):
                        nc.tensor.matmul(
                            cp[:, q * CH:(q + 1) * CH],
                            c1d[:, k * 128:(k + 1) * 128],
                            HN[:, 1 + k + g * 1024 + q * CH:
                               1 + k + g * 1024 + (q + 1) * CH],
                            start=(k == 0), stop=(k == 3))
                sl = slice(g * 1024, (g + 1) * 1024)
                nc.scalar.activation(ZS[:, sl], zp[:], Act.Silu, bias=ip_bz)
                nc.scalar.activation(XC[:, sl], cp[:], Act.Silu, bias=cd_be)

            # ---- dt: precomposed M matmul + exp + ln(1+x) ----
            # g1 first: depends on the last silu group (no table ping-pong),
            # and completes DT half-0 (g0+g1) as early as possible
            for g in ([0, 1, 2, 3] if phases >= 2 else []):
                dp = psA.tile([DI, 1024], f32, tag="mm")
                for q in range(2):
                    nc.tensor.matmul(dp[:, q * CH:(q + 1) * CH], Mdt,
                                     XC[:, g * 1024 + q * CH:
                                        g * 1024 + (q + 1) * CH],
                                     start=True, stop=True)
                esb = spool.tile([DI, 1024], bf16, tag="esb")
                nc.scalar.activation(esb[:], dp[:], Act.Exp, bias=dt_b)
                nc.scalar.activation(DT[:, g * 1024:(g + 1) * 1024], esb[:],
                                     Act.Ln, bias=1.0)
            if phases >= 2:
                nc.vector.tensor_mul(U[:, 0:LH], DT[:, 0:LH], XC[:, 0:LH])
                nc.vector.tensor_mul(U[:, LH:L], DT[:, LH:L], XC[:, LH:L])

            # ---- scan block ----
            # per half: scan phase (dA, B-bcast, dbx, scan -> H_j), then
            # y phase (C-bcast, tmp = H*C, PSUM-accumulated identity matmuls)
            y_in_t = dpool.tile([4, DI, 1024], bf16, tag="yin")
            y_out_t = dpool.tile([4, DI, 1024], bf16, tag="yout")
            for half in range(2):
                hsl = slice(half * LH, (half + 1) * LH)
                Hs = []
                for j in range(NS if phases >= 3 else 0):
                    dA = dapool.tile([DI, LH], f32, tag="dA")
                    nc.scalar.activation(dA[:], DT[:, hsl], Act.Exp,
                                         scale=a_sc[:, j:j + 1])
                    dbx = dbxpool.tile([DI, LH], bf16, tag="dbx")
                    for q in range(2):
                        qsl = slice(half * LH + q * 1024,
                                    half * LH + (q + 1) * 1024)
                        brp = psA.tile([DI, 1024], f32, tag="mm")
                        for r in range(2):
                            nc.tensor.matmul(
                                brp[:, r * CH:(r + 1) * CH],
                                brepT[:, j * 128:(j + 1) * 128],
                                XC[:, qsl][:, r * CH:(r + 1) * CH],
                                start=True, stop=True)
                        rt = B_ROUTE[half * NS + j]
                        if rt == 'dve':
                            nc.vector.tensor_tensor(
                                dbx[:, q * 1024:(q + 1) * 1024],
                                U[:, qsl], brp[:], op=Alu.mult)
                        else:
                            brS = tmppool.tile([DI, 1024], bf16, tag="brS")
                            nc.scalar.copy(brS[:], brp[:])
                            eng = nc.gpsimd if rt == 'ap' else nc.vector
                            eng.tensor_tensor(
                                dbx[:, q * 1024:(q + 1) * 1024],
                                U[:, qsl], brS[:], op=Alu.mult)
                    if half == 1 and j < 2:
                        Hh = bpool.tile([DI, LH], bf16,
                                        tag=("HN0" if j == 0 else "HNT"))
                    else:
                        Hh = hpool.tile([DI, LH], bf16, tag=f"H{j}")
                    init = 0.0 if half == 0 else CAR[:, j:j + 1]
                    nc.vector.tensor_tensor_scan(Hh[:], dA[:], dbx[:], init,
                                                 op0=Alu.mult, op1=Alu.add)
                    if half == 0:
                        nc.gpsimd.tensor_scalar_mul(
                            CAR[:, j:j + 1], Hh[:, LH - 1:LH], 1.0)
                    Hs.append(Hh)
                # y phase for this half
                for q in range(2 if phases >= 3 else 0):
                    qsl = slice(half * LH + q * 1024,
                                half * LH + (q + 1) * 1024)
                    yp = psZ.tile([DI, 1024], f32, tag="z")
                    for j in range(NS):
                        crp = psA.tile([DI, 1024], f32, tag="mm")
                        for r in range(2):
                            nc.tensor.matmul(
                                crp[:, r * CH:(r + 1) * CH],
                                crepT[:, j * 128:(j + 1) * 128],
                                XC[:, qsl][:, r * CH:(r + 1) * CH],
                                start=True, stop=True)
                        tmp = tmppool.tile([DI, 1024], bf16, tag="tmp")
                        rt = C_ROUTE[half * NS + j]
                        if rt == 'dve':
                            nc.vector.scalar_tensor_tensor(
                                tmp[:], crp[:], 1.0,
                                Hs[j][:, q * 1024:(q + 1) * 1024],
                                op0=Alu.mult, op1=Alu.mult)
                        else:
                            crS = tmppool.tile([DI, 1024], bf16, tag="crS")
                            nc.scalar.copy(crS[:], crp[:])
                            eng = nc.gpsimd if rt == 'ap' else nc.vector
                            eng.tensor_tensor(
                                tmp[:], crS[:],
                                Hs[j][:, q * 1024:(q + 1) * 1024],
                                op=Alu.mult)
                        for r in range(2):
                            nc.tensor.matmul(
                                yp[:, r * CH:(r + 1) * CH], ident,
                                tmp[:, r * CH:(r + 1) * CH],
                                start=(j == 0), stop=(j == NS - 1))
                    ye = tmppool.tile([DI, 1024], bf16, tag="ye")
                    nc.scalar.copy(ye[:], yp[:])
                    nc.sync.dma_start(y_in_t[half * 2 + q], ye[:])

            # ---- AllReduce partial y per quarter (pipelines with post) ----
            for half in range(2):
                for q in range(2):
                    qv = y_in_t[half * 2 + q]
                    qo = y_out_t[half * 2 + q]
                    if sim or phases < 3:
                        nc.sync.dma_start(qo, qv)
                    else:
                        nc.gpsimd.collective_compute(
                            "AllReduce", Alu.add, replica_groups=groups,
                            ins=[qv.opt()], outs=[qo.opt()])
                    nc.sync.dma_start(
                        YSUM[:, half * LH + q * 1024:
                             half * LH + (q + 1) * 1024], qo)

            # ---- post: ys = (y + xc*Dp) * silu(z); out = op(ys) + seq ----
            XCD = bpool.tile([DI, L], bf16, name="XCD")
            YS = bpool.tile([DI, L], bf16, name="YS")
            # XCD depends only on XC: scheduler can run it during the scans
            nc.gpsimd.tensor_scalar_mul(XCD[:], XC[:], Dp)
            for half in range(2):
                for qq in range(2):
                    hsl = slice(half * LH + qq * 1024,
                                half * LH + (qq + 1) * 1024)
                    nc.vector.tensor_add(XCD[:, hsl], YSUM[:, hsl],
                                         XCD[:, hsl])
                    nc.vector.tensor_mul(YS[:, hsl], XCD[:, hsl], ZS[:, hsl])
                for ci in range(4):
                    sl = slice(half * LH + ci * CH, half * LH + (ci + 1) * CH)
                    op_ps = psA.tile([C, CH], f32, tag="mm")
                    nc.tensor.matmul(op_ps[:], opT, YS[:, sl],
                                     start=True, stop=True)
                    oc = spool.tile([C, CH], f32, tag="oc")
                    nc.vector.tensor_tensor(oc[:], op_ps[:], SEQ[:, sl],
                                            op=Alu.add)
                    nc.sync.dma_start(out_d[:, sl], oc[:])

    nc.compile()
    return nc


def _host_precompute(inp):
    import ml_dtypes
    f = lambda k: np.asarray(inp[k], np.float32)
    bf = lambda a: np.ascontiguousarray(a.astype(ml_dtypes.bfloat16))
    w1 = f("conv_w")[:, :, 0, 0]              # [out, in]
    wh = f("dwh_w")[:, 0, :, 0]               # [64, 3]
    ww = f("dww_w")[:, 0, 0, :]
    s_bn = f("bn_g") / np.sqrt(f("bn_v") + EPS)
    taps = [
        w1 * (1.0 + wh[:, 1] + ww[:, 1])[None, :],
        w1 * wh[:, 0][None, :],
        w1 * wh[:, 2][None, :],
        w1 * ww[:, 0][None, :],
        w1 * ww[:, 2][None, :],
    ]
    btot = f("conv_b") + w1 @ (f("dwh_b") + f("dww_b"))
    bn_bias = s_bn * (btot - f("bn_m")) + f("bn_b")

    ln_g = f("ln_g"); ln_b = f("ln_b")
    ipw = f("in_proj_w")                       # [256, 64]
    ipw_x = ipw[:DI]; ipw_z = ipw[DI:]
    cdw = f("convd_w")[:, 0, :]                # [128, 4]
    cd_b_eff = f("convd_b") + cdw.sum(1) * (ipw_x @ ln_b)
    ip_b_z = ipw_z @ ln_b
    xpw = f("x_proj_w")                        # [36, 128]
    Mdt = f("dt_proj_w") @ xpw[:DR]            # [128, 128]
    a_full = -np.exp(np.asarray(inp["A_log"], np.float32))

    per_sigma = []
    for sg in range(2):
        s_lo = sg * NS
        cf32 = np.zeros((128, 16), np.float32)
        cf32[:C, 0] = bn_bias
        cf32[:, 1] = ip_b_z
        cf32[:, 2] = cd_b_eff
        cf32[:, 3] = f("dt_proj_b")
        for j in range(NS):
            cf32[:, 4 + j] = a_full[:, s_lo + j]
        cf32[:, 12] = f("Dp")

        cbf = np.zeros((128, 3344), np.float32)
        cbf[:, 0:128] = np.eye(128, dtype=np.float32)
        for k in range(5):
            cbf[:C, 128 + k * 64:128 + (k + 1) * 64] = (taps[k] * s_bn[:, None]).T
        cbf[:C, 448:576] = (ipw_z * ln_g[None, :]).T
        for k in range(4):
            cbf[:C, 576 + k * 128:576 + (k + 1) * 128] = \
                (cdw[:, k][:, None] * ipw_x * ln_g[None, :]).T
        cbf[:, 1088:1216] = Mdt.T
        for j in range(NS):
            cbf[:, 1216 + j] = xpw[DR + s_lo + j]
            cbf[:, 1224 + j] = xpw[DR + DS + s_lo + j]
        cbf[:, 1232:1296] = f("out_proj_w").T
        for j in range(NS):
            cbf[:, 1296 + j * 128:1296 + (j + 1) * 128] = \
                xpw[DR + s_lo + j][:, None]
            cbf[:, 2320 + j * 128:2320 + (j + 1) * 128] = \
                xpw[DR + DS + s_lo + j][:, None]
        per_sigma.append(dict(cf32=cf32, cbf=bf(cbf)))
    return per_sigma


def _pad_image(xb):
    import ml_dtypes
    xp = np.zeros((C, WP, WP), np.float32)
    xp[:, 1:65, 1:65] = xb
    return np.ascontiguousarray(
        xp.reshape(C, XPL).astype(ml_dtypes.bfloat16))


TRACE = False
LAST_EXEC_NS = None
LAST_TRACE_DIR = None


def kernel(**inputs):
    global LAST_EXEC_NS, LAST_TRACE_DIR
    from concourse.bass_utils import run_bass_kernel_spmd

    if "nc" not in _cached:
        _cached["nc"] = _build_program()
    nc = _cached["nc"]

    per_sigma = _host_precompute(inputs)
    x = np.asarray(inputs["x"], np.float32)
    in_maps = []
    for c in range(NCORES):
        b, sg = c // 2, c % 2
        m = dict(per_sigma[sg])
        m["xpad"] = _pad_image(x[b])
        in_maps.append(m)

    kw = {}
    if TRACE:
        import tempfile
        LAST_TRACE_DIR = tempfile.mkdtemp(prefix="bass_trace_")
        kw = dict(trace=True, tmpdir=LAST_TRACE_DIR)
    r = run_bass_kernel_spmd(nc, in_maps, list(range(NCORES)), **kw)
    if r.exec_time_ns is not None:
        LAST_EXEC_NS = r.exec_time_ns
    res = r.results
    out = np.empty((B, C, H, W), np.float32)
    for b in range(B):
        out[b] = np.asarray(res[2 * b]["out_f"], np.float32).reshape(C, H, W)
    return out
